# revision 1
# baseline (speedup 1.0000x reference)
"""KGram MLP seq model (k-gram embedding lookup + 2-layer MLP + vocab projection)
on 8 Trainium2 NeuronCores.

Strategy: data-parallel over the S*B = 4096 token positions (512 rows/core,
cores 0-3 take batch 0, cores 4-7 take batch 1; each core owns a contiguous
span of 512 sequence positions of one batch column).  All weights are
replicated per core (uploaded as bf16).  Per core:

  1. indirect-DMA gather of the (T + K - 1) needed embedding rows from E
     (token-major, [128, D] tiles)
  2. PE transpose -> feature-major G^T tiles [128, T+K-1]
  3. h1^T = silu(W1^T x^T + b1) where the three K-blocks of x^T are just
     shifted column windows of G^T (the k-gram windows overlap)
  4. h2^T = silu(W2^T h1^T + b2)
  5. logits^T = Wout^T h2^T + bout, streamed over vocab in 1024-col groups
     (bf16 weights, f32 PSUM accumulate, f32 output)

Host reassembles out[s, b, :] from the per-core logits^T shards.
"""

import math

import numpy as np
import ml_dtypes

import concourse.bass as bass
import concourse.mybir as mybir
import concourse.tile as tile
from concourse import bacc
from concourse.bass_utils import run_bass_kernel_spmd

P = 128
NCORES = 8

# Full-problem constants (hardcoded; kernel.py must be self-contained)
VOCAB = 50257
EMBED = 1024
SEQ = 2048
BATCH = 2
KGRAM = 3
VPAD = 50304  # 393 * 128
MGROUP = 1024  # vocab columns per Wout streaming group

_nc_cache: dict = {}


def _build(V, D, KC, T, VP, MG):
    """Build the single-core Bass graph (SPMD: same graph on all cores)."""
    DK = D // P
    TW = T + KC - 1
    NG = math.ceil(TW / P)
    TWPAD = NG * P
    NM = VP // P
    f32 = mybir.dt.float32
    bf16 = mybir.dt.bfloat16
    i32 = mybir.dt.int32
    AF = mybir.ActivationFunctionType

    nc = bacc.Bacc()

    E_d = nc.declare_dram_parameter("E", [V, D], bf16, isOutput=False)
    W1_d = nc.declare_dram_parameter("W1", [KC * D, D], bf16, isOutput=False)
    W2_d = nc.declare_dram_parameter("W2", [D, D], bf16, isOutput=False)
    Wo_d = nc.declare_dram_parameter("Wo", [D, VP], bf16, isOutput=False)
    b1_d = nc.declare_dram_parameter("b1", [P, DK], f32, isOutput=False)
    b2_d = nc.declare_dram_parameter("b2", [P, DK], f32, isOutput=False)
    bo_d = nc.declare_dram_parameter("bo", [P, NM], f32, isOutput=False)
    tok_d = nc.declare_dram_parameter("toks", [P, NG], i32, isOutput=False)
    out_d = nc.declare_dram_parameter("out", [VP, T], f32, isOutput=True)

    with tile.TileContext(nc) as tc:
        with (
            tc.tile_pool(name="const", bufs=1) as cpool,
            tc.tile_pool(name="gath", bufs=5) as gpool,
            tc.tile_pool(name="gt", bufs=1) as gtpool,
            tc.tile_pool(name="w", bufs=1) as wpool,
            tc.tile_pool(name="h", bufs=1) as hpool,
            tc.tile_pool(name="wo", bufs=2) as wopool,
            tc.tile_pool(name="ot", bufs=4) as opool,
            tc.tile_pool(name="psA", bufs=2, space="PSUM") as psA,
            tc.tile_pool(name="psB", bufs=6, space="PSUM") as psB,
        ):
            # token indices first so the gathers start immediately
            tok_s = cpool.tile([P, NG], i32, tag="tok")
            nc.sync.dma_start(tok_s[:], tok_d[:])

            # --- embedding gather (token-major), all gathers queued up front ---
            # Only TW of the NG*P index slots are real tokens; size the last
            # gather down to the next multiple of 16 (XBAR row granularity).
            gts = [gtpool.tile([P, TWPAD], bf16, tag=f"gt{f}", name=f"gt{f}") for f in range(DK)]
            gtiles = []
            grows = []
            for g in range(NG):
                rows = min(P, TW - g * P)
                rows = ((rows + 15) // 16) * 16  # pad to XBAR granularity
                gtile = gpool.tile([P, D], bf16, tag="g", name=f"g{g}")
                nc.gpsimd.indirect_dma_start(
                    out=gtile[:rows, :],
                    out_offset=None,
                    in_=E_d[:],
                    in_offset=bass.IndirectOffsetOnAxis(
                        ap=tok_s[:rows, g : g + 1], axis=0
                    ),
                )
                gtiles.append(gtile)
                grows.append(rows)

            # PE warmup: burn the HAM cold window on dummy matmuls while the
            # gathers are in flight, so the real matmul stream starts at 2.4 GHz.
            warm = cpool.tile([P, P], bf16, tag="warm")
            nc.vector.memset(warm[:], 0.5)
            WN = min(P, T)
            warm_ps = psA.tile([P, T], f32, tag="mlp", name="warm_ps")
            for _ in range(40):
                nc.tensor.matmul(
                    warm_ps[:, :WN], lhsT=warm[:], rhs=warm[:, :WN],
                    start=True, stop=True,
                )

            b1_s = cpool.tile([P, DK], f32, tag="b1")
            nc.sync.dma_start(b1_s[:], b1_d[:])
            b2_s = cpool.tile([P, DK], f32, tag="b2")
            nc.sync.dma_start(b2_s[:], b2_d[:])
            bo_s = cpool.tile([P, NM], f32, tag="bo")
            nc.sync.dma_start(bo_s[:], bo_d[:])

            # --- transpose to feature-major G^T via the DMA xbar (scalar HWDGE
            # queue; keeps the PE free and the sync queue on weight loads) ---
            for g in range(NG):
                for f in range(DK):
                    nc.scalar.dma_start_transpose(
                        gts[f][:, g * P : g * P + grows[g]],
                        gtiles[g][: grows[g], f * P : (f + 1) * P],
                    )
                # keep the PE warm between gather arrivals
                for _ in range(25):
                    nc.tensor.matmul(
                        warm_ps[:, :WN], lhsT=warm[:], rhs=warm[:, :WN],
                        start=True, stop=True,
                    )

            # --- MLP layer 1: h1^T = silu(W1^T x^T + b1) ---
            w1s = []
            for kc in range(KC * DK):
                t = wpool.tile([P, D], bf16, tag=f"w1_{kc}", name=f"w1_{kc}")
                nc.sync.dma_start(t[:], W1_d[kc * P : (kc + 1) * P, :])
                w1s.append(t)
            h1 = [hpool.tile([P, T], bf16, tag=f"h1_{m}", name=f"h1_{m}") for m in range(DK)]
            for m in range(DK):
                ps = psA.tile([P, T], f32, tag="mlp")
                n = 0
                for i in range(KC):
                    for k8 in range(DK):
                        kc = i * DK + k8
                        nc.tensor.matmul(
                            ps[:],
                            lhsT=w1s[kc][:, m * P : (m + 1) * P],
                            rhs=gts[k8][:, i : i + T],
                            start=(n == 0),
                            stop=(n == KC * DK - 1),
                        )
                        n += 1
                nc.scalar.activation(h1[m][:], ps[:], AF.Silu, bias=b1_s[:, m : m + 1])

            # --- MLP layer 2: h2^T = silu(W2^T h1^T + b2) ---
            w2s = []
            for kc in range(DK):
                t = wpool.tile([P, D], bf16, tag=f"w2_{kc}", name=f"w2_{kc}")
                nc.sync.dma_start(t[:], W2_d[kc * P : (kc + 1) * P, :])
                w2s.append(t)
            h2 = [hpool.tile([P, T], bf16, tag=f"h2_{m}", name=f"h2_{m}") for m in range(DK)]
            for m in range(DK):
                ps = psA.tile([P, T], f32, tag="mlp")
                for k8 in range(DK):
                    nc.tensor.matmul(
                        ps[:],
                        lhsT=w2s[k8][:, m * P : (m + 1) * P],
                        rhs=h1[k8][:],
                        start=(k8 == 0),
                        stop=(k8 == DK - 1),
                    )
                nc.scalar.activation(h2[m][:], ps[:], AF.Silu, bias=b2_s[:, m : m + 1])

            # --- vocab projection: logits^T = Wout^T h2^T + bout ---
            c0 = 0
            while c0 < VP:
                cols = min(MG, VP - c0)
                wos = []
                for k8 in range(DK):
                    t = wopool.tile([P, MG], bf16, tag=f"wo{k8}", name=f"wo{k8}")
                    nc.sync.dma_start(
                        t[:, :cols], Wo_d[k8 * P : (k8 + 1) * P, c0 : c0 + cols]
                    )
                    wos.append(t)
                for m in range(cols // P):
                    ps = psB.tile([P, T], f32, tag="proj")
                    for k8 in range(DK):
                        nc.tensor.matmul(
                            ps[:],
                            lhsT=wos[k8][:, m * P : (m + 1) * P],
                            rhs=h2[k8][:],
                            start=(k8 == 0),
                            stop=(k8 == DK - 1),
                        )
                    ot = opool.tile([P, T], f32, tag="ot")
                    mi = (c0 + m * P) // P
                    nc.scalar.activation(
                        ot[:], ps[:], AF.Identity, bias=bo_s[:, mi : mi + 1]
                    )
                    nc.sync.dma_start(out_d[c0 + m * P : c0 + (m + 1) * P, :], ot[:])
                c0 += cols

    nc.finalize()
    return nc


def _get_nc(V, D, KC, T, VP, MG):
    key = (V, D, KC, T, VP, MG)
    if key not in _nc_cache:
        _nc_cache[key] = _build(V, D, KC, T, VP, MG)
    return _nc_cache[key]


def _run(tokens, E, W1, b1, W2, b2, Wout, bout, V, D, KC, VP, MG, trace=False):
    """tokens: (S, B) int32.  Returns (S, B, V) f32 logits (and results obj)."""
    bf16 = ml_dtypes.bfloat16
    S, B = tokens.shape
    cpb = NCORES // B  # cores per batch column
    T = S // cpb
    DK = D // P
    TW = T + KC - 1
    NG = math.ceil(TW / P)
    TWPAD = NG * P
    NM = VP // P

    E_b = E.astype(bf16)
    W1_b = W1.astype(bf16)
    W2_b = W2.astype(bf16)
    Wo_b = np.zeros((D, VP), dtype=bf16)
    Wo_b[:, :V] = Wout.astype(bf16)
    b1t = np.ascontiguousarray(b1.reshape(DK, P).T.astype(np.float32))
    b2t = np.ascontiguousarray(b2.reshape(DK, P).T.astype(np.float32))
    bo_p = np.zeros(VP, dtype=np.float32)
    bo_p[:V] = bout
    bot = np.ascontiguousarray(bo_p.reshape(NM, P).T)

    nc = _get_nc(V, D, KC, T, VP, MG)

    in_maps = []
    for c in range(NCORES):
        b, chunk = divmod(c, cpb)
        s0 = chunk * T
        pad = np.zeros(TWPAD, dtype=np.int32)
        lo = max(0, s0 - (KC - 1))
        seg = tokens[lo : s0 + T, b]
        start = (KC - 1) - (s0 - lo)
        pad[start : start + seg.size] = seg
        tok2d = np.ascontiguousarray(pad.reshape(NG, P).T)
        in_maps.append(
            {
                "E": E_b,
                "W1": W1_b,
                "W2": W2_b,
                "Wo": Wo_b,
                "b1": b1t,
                "b2": b2t,
                "bo": bot,
                "toks": tok2d,
            }
        )

    kres = run_bass_kernel_spmd(nc, in_maps, list(range(NCORES)), trace=trace)
    res = kres.results

    out = np.empty((S, B, V), dtype=np.float32)
    for c in range(NCORES):
        b, chunk = divmod(c, cpb)
        s0 = chunk * T
        out[s0 : s0 + T, b, :] = res[c]["out"][:V, :].T
    return out, kres


def kernel(**inputs):
    tokens = np.asarray(inputs["tokens_seq"]).astype(np.int32)
    E = np.asarray(inputs["E"], dtype=np.float32)
    W1 = np.asarray(inputs["W1"], dtype=np.float32)
    b1 = np.asarray(inputs["b1"], dtype=np.float32)
    W2 = np.asarray(inputs["W2"], dtype=np.float32)
    b2 = np.asarray(inputs["b2"], dtype=np.float32)
    Wout = np.asarray(inputs["Wout"], dtype=np.float32)
    bout = np.asarray(inputs["bout"], dtype=np.float32)
    out, _ = _run(
        tokens, E, W1, b1, W2, b2, Wout, bout,
        V=VOCAB, D=EMBED, KC=KGRAM, VP=VPAD, MG=MGROUP,
    )
    return out

